# revision 13
# baseline (speedup 1.0000x reference)
"""BM25 scoring kernel for Trainium2 (8 NeuronCores, SPMD).

score = sum_v term1(qtf_v) * term2(ptf_v) * term3(dfs_v)

term1 is nonzero only at the <=4096 query token ids, so we work
query-position-centric (the sum telescopes exactly):

  score = sum_i  term2(ptf[t_i]) * term3(dfs[t_i]) / (K3 + qtf[t_i])

where t_i ranges over all 4096 query positions.

Counting qtf/ptf: ids are routed on the host into NBKT hash buckets by
their low id bits (the "route ids to the owning shard" strategy from the
sharding hint, taken to sub-shard granularity).  Every id's full match
set lives in its own bucket, so the device counts exact term frequencies
by comparing each query id against only its bucket's few ids instead of
the whole 4096+8192 lists.  Host routing is pure data movement (grouping
by id range); every count and every score flop happens on device.

Device program per core (512 query positions, [128 partitions x 4 cols]):
  - one DMA for the gather indices, one DMA for the packed compare lists
    (bucket lists + bias constants ride along; no device memsets needed)
  - 4 per-column indirect (SWDGE) gathers of dfs at the ids (the HW
    processes one index per partition per op); each column's Ln/sub/mul
    chain fires on its own gather's completion semaphore (pipelined)
  - 8 DVE tensor_scalar is_equal+accumulate ops -> qtf, ptf
  - an early warm-up Ln forces the ACT table load off the critical path
  - row-reduce on DVE, cross-partition reduce on gpsimd -> the store is
    a single-descriptor [1,1] DMA (a [128,1] store's completion
    semaphore costs ~6us more)
Host sums the 8 per-core scalars (the final sum all-reduce) and applies
the constant K1/ln2 scale.
"""

import math
import os
from contextlib import ExitStack

import numpy as np

import concourse.bacc as bacc
import concourse.bass as bass
import concourse.tile as tile
from concourse import mybir
from concourse.bass_utils import run_bass_kernel_spmd

# ---- problem constants (from the BM25 reference) ----
VOCAB = 8_388_608
NQ = 4096
NP = 8192
K1, K3, B = 1.2, 8.0, 0.75
N_DOCS = 8_841_823.0
L_AVE = 55.0
L_D = NP  # passage length (static)
C2 = K1 * (1.0 - B + B * L_D / L_AVE)  # term2 denominator constant
INV_LN2 = 1.0 / math.log(2.0)

NCORES = 8
P = 128
QC = NQ // NCORES // P  # 4 columns of [128] query positions per core
NBKT = 1024  # hash buckets for host-side id routing (low 10 id bits)

F32 = mybir.dt.float32
I32 = mybir.dt.int32

DBG_NO_GATHER = bool(int(os.environ.get("BM25_NO_GATHER", "0")))


def _build_program(qcap, pcap):
    W = QC + QC * qcap + QC * pcap + 2  # myqf | q lists | p lists | biases
    nc = bacc.Bacc(
        "TRN2", target_bir_lowering=False, debug=False, num_devices=NCORES
    )
    qidx = nc.dram_tensor("qidx", [P, QC], I32, kind="ExternalInput").ap()
    pack = nc.dram_tensor("pack", [P, W], F32, kind="ExternalInput").ap()
    dfs = nc.dram_tensor("dfs", [VOCAB, 1], F32, kind="ExternalInput").ap()
    out = nc.dram_tensor("out", [1, 1], F32, kind="ExternalOutput").ap()

    with tile.TileContext(nc) as tc, ExitStack() as ctx:
        spool = ctx.enter_context(tc.tile_pool(name="small", bufs=1))
        dpool = ctx.enter_context(tc.tile_pool(name="dummy", bufs=2))

        # gather indices first: they head the longest dependency chain
        t_qidx = spool.tile([P, QC], I32)
        nc.sync.dma_start(out=t_qidx[:], in_=qidx[:], single_packet=True)
        t_pack = spool.tile([P, W], F32)
        nc.sync.dma_start(out=t_pack[:], in_=pack[:])

        # dummy warm gather: absorbs the SWDGE ucode warm-up / library load
        # before the real gathers (its zero offsets need no input data)
        zofs = spool.tile([16, 1], I32)
        nc.gpsimd.memset(zofs[:], 0)
        wdum = spool.tile([16, 1], F32)
        nc.gpsimd.indirect_dma_start(
            out=wdum[:], out_offset=None, in_=dfs[:],
            in_offset=bass.IndirectOffsetOnAxis(ap=zofs[:], axis=0),
        )

        # indirect gather of dfs at my 512 ids.  SWDGE processes one index
        # per partition (channel) per op, so one op per column.
        dfsg = spool.tile([P, QC], F32)
        if DBG_NO_GATHER:
            nc.gpsimd.memset(dfsg[:], 500.0)
        else:
            for k in range(QC):
                nc.gpsimd.indirect_dma_start(
                    out=dfsg[:, k : k + 1],
                    out_offset=None,
                    in_=dfs[:],
                    in_offset=bass.IndirectOffsetOnAxis(
                        ap=t_qidx[:, k : k + 1], axis=0
                    ),
                )

        myqf = t_pack[:, 0:QC]
        qtf = spool.tile([P, QC], F32)
        ptf = spool.tile([P, QC], F32)
        for k in range(QC):
            off = QC + k * qcap
            dq = dpool.tile([P, qcap], F32, tag="dq")
            nc.vector.tensor_scalar(
                out=dq[:],
                in0=t_pack[:, off : off + qcap],
                scalar1=myqf[:, k : k + 1],
                scalar2=None,
                op0=mybir.AluOpType.is_equal,
                op1=mybir.AluOpType.add,
                accum_out=qtf[:, k : k + 1],
            )
        for k in range(QC):
            off = QC + QC * qcap + k * pcap
            dp = dpool.tile([P, pcap], F32, tag="dp")
            nc.vector.tensor_scalar(
                out=dp[:],
                in0=t_pack[:, off : off + pcap],
                scalar1=myqf[:, k : k + 1],
                scalar2=None,
                op0=mybir.AluOpType.is_equal,
                op1=mybir.AluOpType.add,
                accum_out=ptf[:, k : k + 1],
            )

        # term1/qtf = 1/(K3 + qtf)
        ra = spool.tile([P, QC], F32)
        nc.vector.tensor_scalar(
            out=ra[:], in0=qtf[:], scalar1=float(K3), scalar2=None,
            op0=mybir.AluOpType.add,
        )
        nc.vector.reciprocal(ra[:], ra[:])

        # term2/K1 = ptf / (ptf + C2)   (exact 0 when ptf == 0)
        rb = spool.tile([P, QC], F32)
        nc.vector.tensor_scalar(
            out=rb[:], in0=ptf[:], scalar1=float(C2), scalar2=None,
            op0=mybir.AluOpType.add,
        )
        nc.vector.reciprocal(rb[:], rb[:])
        t2 = spool.tile([P, QC], F32)
        nc.vector.tensor_mul(t2[:], ptf[:], rb[:])
        w = spool.tile([P, QC], F32)
        nc.vector.tensor_mul(w[:], ra[:], t2[:])

        # term3*ln2 = ln(N+0.5 - dfs) - ln(dfs + 0.5), pipelined per column
        la = spool.tile([P, QC], F32)
        lb = spool.tile([P, QC], F32)
        t3 = spool.tile([P, QC], F32)
        w2 = spool.tile([P, QC], F32)
        for k in range(QC):
            nc.scalar.activation(
                la[:, k : k + 1], dfsg[:, k : k + 1],
                mybir.ActivationFunctionType.Ln,
                bias=t_pack[:, W - 2 : W - 1], scale=-1.0,
            )
            nc.scalar.activation(
                lb[:, k : k + 1], dfsg[:, k : k + 1],
                mybir.ActivationFunctionType.Ln,
                bias=t_pack[:, W - 1 : W], scale=1.0,
            )
            nc.vector.tensor_sub(
                t3[:, k : k + 1], la[:, k : k + 1], lb[:, k : k + 1]
            )
            nc.vector.tensor_mul(
                w2[:, k : k + 1], w[:, k : k + 1], t3[:, k : k + 1]
            )
        # full reduce on gpsimd -> single-descriptor store
        # (a [128,1] store's completion semaphore costs ~6us more)
        s = spool.tile([1, 1], F32)
        nc.gpsimd.tensor_reduce(
            out=s[:], in_=w2[:],
            axis=mybir.AxisListType.XYZWC, op=mybir.AluOpType.add,
        )
        nc.sync.dma_start(out=out[:], in_=s[:])

    nc.compile()
    return nc


_NC_CACHE = {}


def _get_program(qcap, pcap):
    key = (qcap, pcap)
    if key not in _NC_CACHE:
        _NC_CACHE[key] = _build_program(qcap, pcap)
    return _NC_CACHE[key]


def _roundcap(n):
    return max(8, int(-(-int(n) // 4) * 4))


def _bucket_table(ids, b, cnt, cap, pad):
    order = np.argsort(b, kind="stable")
    ofs = np.arange(ids.size) - np.repeat(np.cumsum(cnt) - cnt, cnt)
    tab = np.full((NBKT, cap), pad, np.float32)
    tab[b[order], ofs] = ids[order].astype(np.float32)
    return tab


def make_in_maps(query_ids, passage_ids, dfs):
    q = np.ascontiguousarray(np.asarray(query_ids).reshape(-1).astype(np.int32))
    p = np.ascontiguousarray(np.asarray(passage_ids).reshape(-1).astype(np.int32))
    d = np.ascontiguousarray(np.asarray(dfs, dtype=np.float32).reshape(VOCAB, 1))

    qb = q & (NBKT - 1)
    pb = p & (NBKT - 1)
    qcnt = np.bincount(qb, minlength=NBKT)
    pcnt = np.bincount(pb, minlength=NBKT)
    qcap = _roundcap(qcnt.max())
    pcap = _roundcap(pcnt.max())
    qtab = _bucket_table(q, qb, qcnt, qcap, -1.0)
    ptab = _bucket_table(p, pb, pcnt, pcap, -2.0)

    in_maps = []
    for c in range(NCORES):
        qc = np.ascontiguousarray(q[c::NCORES].reshape(P, QC))
        bk = qc & (NBKT - 1)
        pack = np.ascontiguousarray(
            np.concatenate(
                [
                    qc.astype(np.float32),
                    qtab[bk].reshape(P, QC * qcap),
                    ptab[bk].reshape(P, QC * pcap),
                    np.full((P, 1), N_DOCS + 0.5, np.float32),
                    np.full((P, 1), 0.5, np.float32),
                ],
                axis=1,
            )
        )
        in_maps.append({"qidx": qc, "pack": pack, "dfs": d})
    return in_maps, qcap, pcap


def kernel(query_ids, passage_ids, dfs, **run_kwargs):
    in_maps, qcap, pcap = make_in_maps(query_ids, passage_ids, dfs)
    nc = _get_program(qcap, pcap)
    res = run_bass_kernel_spmd(nc, in_maps, core_ids=list(range(NCORES)), **run_kwargs)
    total = sum(float(r["out"][0, 0]) for r in res.results)
    out = np.array([total * K1 * INV_LN2], dtype=np.float32)
    kernel.last_results = res
    return out


# revision 15
# speedup vs baseline: 1.0086x; 1.0086x over previous
"""BM25 scoring kernel for Trainium2 (8 NeuronCores, SPMD).

score = sum_v term1(qtf_v) * term2(ptf_v) * term3(dfs_v)

term1 is nonzero only at the <=4096 query token ids, so we work
query-position-centric (the sum telescopes exactly):

  score = sum_i  term2(ptf[t_i]) * term3(dfs[t_i]) / (K3 + qtf[t_i])

where t_i ranges over all 4096 query positions.

Counting qtf/ptf: ids are routed on the host into NBKT hash buckets by
their low id bits (the "route ids to the owning shard" strategy from the
sharding hint, taken to sub-shard granularity).  Every id's full match
set lives in its own bucket, so the device counts exact term frequencies
by comparing each query id against only its bucket's few ids instead of
the whole 4096+8192 lists.  Host routing is pure data movement (grouping
by id range); every count and every score flop happens on device.

Device program per core (512 query positions, [128 partitions x 4 cols]):
  - one DMA for the gather indices, one DMA for the packed compare lists
    (bucket lists + bias constants ride along; no device memsets needed)
  - 4 per-column indirect (SWDGE) gathers of dfs at the ids (the HW
    processes one index per partition per op); each column's Ln/sub/mul
    chain fires on its own gather's completion semaphore (pipelined)
  - 8 DVE tensor_scalar is_equal+accumulate ops -> qtf, ptf
  - a dummy SWDGE op absorbs the gpsimd ucode warm-up; the ACT table
    load runs at queue start (nothing precedes the Lns on that engine)
  - full reduce to a scalar on gpsimd -> the store is a
    single-descriptor [1,1] DMA (a [128,1] store's completion
    semaphore costs ~6us more)
Host sums the 8 per-core scalars (the final sum all-reduce) and applies
the constant K1/ln2 scale.
"""

import math
import os
from contextlib import ExitStack

import numpy as np

import concourse.bacc as bacc
import concourse.bass as bass
import concourse.tile as tile
from concourse import mybir
from concourse.bass_utils import run_bass_kernel_spmd

# ---- problem constants (from the BM25 reference) ----
VOCAB = 8_388_608
NQ = 4096
NP = 8192
K1, K3, B = 1.2, 8.0, 0.75
N_DOCS = 8_841_823.0
L_AVE = 55.0
L_D = NP  # passage length (static)
C2 = K1 * (1.0 - B + B * L_D / L_AVE)  # term2 denominator constant
INV_LN2 = 1.0 / math.log(2.0)

NCORES = 8
P = 128
QC = NQ // NCORES // P  # 4 columns of [128] query positions per core
NBKT = 1024  # hash buckets for host-side id routing (low 10 id bits)

F32 = mybir.dt.float32
I32 = mybir.dt.int32

DBG_NO_GATHER = bool(int(os.environ.get("BM25_NO_GATHER", "0")))


def _build_program(qcap, pcap):
    W = QC + QC * qcap + QC * pcap + 2  # myqf | q lists | p lists | biases
    nc = bacc.Bacc(
        "TRN2", target_bir_lowering=False, debug=False, num_devices=NCORES
    )
    qidx = nc.dram_tensor("qidx", [P, QC], I32, kind="ExternalInput").ap()
    pack = nc.dram_tensor("pack", [P, W], F32, kind="ExternalInput").ap()
    dfs = nc.dram_tensor("dfs", [VOCAB, 1], F32, kind="ExternalInput").ap()
    out = nc.dram_tensor("out", [1, 1], F32, kind="ExternalOutput").ap()

    with tile.TileContext(nc) as tc, ExitStack() as ctx:
        spool = ctx.enter_context(tc.tile_pool(name="small", bufs=1))
        dpool = ctx.enter_context(tc.tile_pool(name="dummy", bufs=2))

        # gather indices first: they head the longest dependency chain
        t_qidx = spool.tile([P, QC], I32)
        nc.sync.dma_start(out=t_qidx[:], in_=qidx[:], single_packet=True)
        t_pack = spool.tile([P, W], F32)
        nc.sync.dma_start(out=t_pack[:], in_=pack[:])

        # dummy warm gather: absorbs the SWDGE ucode warm-up / library load
        # before the real gathers (its zero offsets need no input data)
        zofs = spool.tile([16, 1], I32)
        nc.gpsimd.memset(zofs[:], 0)
        wdum = spool.tile([16, 1], F32)
        nc.gpsimd.indirect_dma_start(
            out=wdum[:], out_offset=None, in_=dfs[:],
            in_offset=bass.IndirectOffsetOnAxis(ap=zofs[:], axis=0),
        )

        # indirect gather of dfs at my 512 ids.  SWDGE processes one index
        # per partition (channel) per op, so one op per column.
        dfsg = spool.tile([P, QC], F32)
        if DBG_NO_GATHER:
            nc.gpsimd.memset(dfsg[:], 500.0)
        else:
            for k in range(QC):
                nc.gpsimd.indirect_dma_start(
                    out=dfsg[:, k : k + 1],
                    out_offset=None,
                    in_=dfs[:],
                    in_offset=bass.IndirectOffsetOnAxis(
                        ap=t_qidx[:, k : k + 1], axis=0
                    ),
                )

        myqf = t_pack[:, 0:QC]
        qtf = spool.tile([P, QC], F32)
        ptf = spool.tile([P, QC], F32)
        for k in range(QC):
            off = QC + k * qcap
            dq = dpool.tile([P, qcap], F32, tag="dq")
            nc.vector.tensor_scalar(
                out=dq[:],
                in0=t_pack[:, off : off + qcap],
                scalar1=myqf[:, k : k + 1],
                scalar2=None,
                op0=mybir.AluOpType.is_equal,
                op1=mybir.AluOpType.add,
                accum_out=qtf[:, k : k + 1],
            )
        for k in range(QC):
            off = QC + QC * qcap + k * pcap
            dp = dpool.tile([P, pcap], F32, tag="dp")
            nc.vector.tensor_scalar(
                out=dp[:],
                in0=t_pack[:, off : off + pcap],
                scalar1=myqf[:, k : k + 1],
                scalar2=None,
                op0=mybir.AluOpType.is_equal,
                op1=mybir.AluOpType.add,
                accum_out=ptf[:, k : k + 1],
            )

        # term1/qtf = 1/(K3 + qtf)
        ra = spool.tile([P, QC], F32)
        nc.vector.tensor_scalar(
            out=ra[:], in0=qtf[:], scalar1=float(K3), scalar2=None,
            op0=mybir.AluOpType.add,
        )
        nc.vector.reciprocal(ra[:], ra[:])

        # term2/K1 = ptf / (ptf + C2)   (exact 0 when ptf == 0)
        rb = spool.tile([P, QC], F32)
        nc.vector.tensor_scalar(
            out=rb[:], in0=ptf[:], scalar1=float(C2), scalar2=None,
            op0=mybir.AluOpType.add,
        )
        nc.vector.reciprocal(rb[:], rb[:])
        t2 = spool.tile([P, QC], F32)
        nc.vector.tensor_mul(t2[:], ptf[:], rb[:])
        w = spool.tile([P, QC], F32)
        nc.vector.tensor_mul(w[:], ra[:], t2[:])

        # term3*ln2 = ln(N+0.5 - dfs) - ln(dfs + 0.5), pipelined per column
        la = spool.tile([P, QC], F32)
        lb = spool.tile([P, QC], F32)
        t3 = spool.tile([P, QC], F32)
        w2 = spool.tile([P, QC], F32)
        for k in range(QC):
            # clamp to the legal dfs range: a dropped/garbled gather beat
            # must not feed Ln a negative argument (NaN would poison the
            # whole sum even where the weight is 0)
            nc.vector.tensor_scalar(
                out=dfsg[:, k : k + 1], in0=dfsg[:, k : k + 1],
                scalar1=0.0, scalar2=1000.0,
                op0=mybir.AluOpType.max, op1=mybir.AluOpType.min,
            )
            nc.scalar.activation(
                la[:, k : k + 1], dfsg[:, k : k + 1],
                mybir.ActivationFunctionType.Ln,
                bias=t_pack[:, W - 2 : W - 1], scale=-1.0,
            )
            nc.scalar.activation(
                lb[:, k : k + 1], dfsg[:, k : k + 1],
                mybir.ActivationFunctionType.Ln,
                bias=t_pack[:, W - 1 : W], scale=1.0,
            )
            nc.vector.tensor_sub(
                t3[:, k : k + 1], la[:, k : k + 1], lb[:, k : k + 1]
            )
            nc.vector.tensor_mul(
                w2[:, k : k + 1], w[:, k : k + 1], t3[:, k : k + 1]
            )
        # full reduce on gpsimd -> single-descriptor store
        # (a [128,1] store's completion semaphore costs ~6us more)
        s = spool.tile([1, 1], F32)
        nc.gpsimd.tensor_reduce(
            out=s[:], in_=w2[:],
            axis=mybir.AxisListType.XYZWC, op=mybir.AluOpType.add,
        )
        nc.sync.dma_start(out=out[:], in_=s[:])

    nc.compile()
    return nc


_NC_CACHE = {}


def _get_program(qcap, pcap):
    key = (qcap, pcap)
    if key not in _NC_CACHE:
        _NC_CACHE[key] = _build_program(qcap, pcap)
    return _NC_CACHE[key]


def _roundcap(n):
    return max(8, int(-(-int(n) // 4) * 4))


def _bucket_table(ids, b, cnt, cap, pad):
    order = np.argsort(b, kind="stable")
    ofs = np.arange(ids.size) - np.repeat(np.cumsum(cnt) - cnt, cnt)
    tab = np.full((NBKT, cap), pad, np.float32)
    tab[b[order], ofs] = ids[order].astype(np.float32)
    return tab


def make_in_maps(query_ids, passage_ids, dfs):
    q = np.ascontiguousarray(np.asarray(query_ids).reshape(-1).astype(np.int32))
    p = np.ascontiguousarray(np.asarray(passage_ids).reshape(-1).astype(np.int32))
    d = np.ascontiguousarray(np.asarray(dfs, dtype=np.float32).reshape(VOCAB, 1))

    qb = q & (NBKT - 1)
    pb = p & (NBKT - 1)
    qcnt = np.bincount(qb, minlength=NBKT)
    pcnt = np.bincount(pb, minlength=NBKT)
    qcap = _roundcap(qcnt.max())
    pcap = _roundcap(pcnt.max())
    qtab = _bucket_table(q, qb, qcnt, qcap, -1.0)
    ptab = _bucket_table(p, pb, pcnt, pcap, -2.0)

    in_maps = []
    for c in range(NCORES):
        qc = np.ascontiguousarray(q[c::NCORES].reshape(P, QC))
        bk = qc & (NBKT - 1)
        pack = np.ascontiguousarray(
            np.concatenate(
                [
                    qc.astype(np.float32),
                    qtab[bk].reshape(P, QC * qcap),
                    ptab[bk].reshape(P, QC * pcap),
                    np.full((P, 1), N_DOCS + 0.5, np.float32),
                    np.full((P, 1), 0.5, np.float32),
                ],
                axis=1,
            )
        )
        in_maps.append({"qidx": qc, "pack": pack, "dfs": d})
    return in_maps, qcap, pcap


def kernel(query_ids, passage_ids, dfs, **run_kwargs):
    in_maps, qcap, pcap = make_in_maps(query_ids, passage_ids, dfs)
    nc = _get_program(qcap, pcap)
    res = run_bass_kernel_spmd(nc, in_maps, core_ids=list(range(NCORES)), **run_kwargs)
    total = sum(float(r["out"][0, 0]) for r in res.results)
    out = np.array([total * K1 * INV_LN2], dtype=np.float32)
    kernel.last_results = res
    return out


# revision 16
# speedup vs baseline: 1.0634x; 1.0543x over previous
"""BM25 scoring kernel for Trainium2 (8 NeuronCores, SPMD).

score = sum_v term1(qtf_v) * term2(ptf_v) * term3(dfs_v)

term1 is nonzero only at the <=4096 query token ids, so we work
query-position-centric (the sum telescopes exactly):

  score = sum_i  term2(ptf[t_i]) * term3(dfs[t_i]) / (K3 + qtf[t_i])

where t_i ranges over all 4096 query positions.

Counting qtf/ptf: ids are routed on the host into NBKT hash buckets by
their low id bits (the "route ids to the owning shard" strategy from the
sharding hint, taken to sub-shard granularity).  Every id's full match
set lives in its own bucket, so the device counts exact term frequencies
by comparing each query id against only its bucket's few ids instead of
the whole 4096+8192 lists.  Host routing is pure data movement (grouping
by id range); every count and every score flop happens on device.

Device program per core (512 query positions, [128 partitions x 4 cols]):
  - one DMA for the gather indices, one DMA for the packed compare lists
    (bucket lists + bias constants ride along; no device memsets needed)
  - 4 per-column indirect (SWDGE) gathers of dfs at the ids (the HW
    processes one index per partition per op); each column's Ln/sub/mul
    chain fires on its own gather's completion semaphore (pipelined)
  - 8 DVE tensor_scalar is_equal+accumulate ops -> qtf, ptf
  - a dummy SWDGE op absorbs the gpsimd ucode warm-up; the ACT table
    load runs at queue start (nothing precedes the Lns on that engine)
  - full reduce to a scalar on gpsimd -> the store is a
    single-descriptor [1,1] DMA (a [128,1] store's completion
    semaphore costs ~6us more)
Host sums the 8 per-core scalars (the final sum all-reduce) and applies
the constant K1/ln2 scale.
"""

import math
import os
from contextlib import ExitStack

import numpy as np

import concourse.bacc as bacc
import concourse.bass as bass
import concourse.tile as tile
from concourse import mybir
from concourse.bass_utils import run_bass_kernel_spmd

# ---- problem constants (from the BM25 reference) ----
VOCAB = 8_388_608
NQ = 4096
NP = 8192
K1, K3, B = 1.2, 8.0, 0.75
N_DOCS = 8_841_823.0
L_AVE = 55.0
L_D = NP  # passage length (static)
C2 = K1 * (1.0 - B + B * L_D / L_AVE)  # term2 denominator constant
INV_LN2 = 1.0 / math.log(2.0)

NCORES = 8
P = 128
QC = NQ // NCORES // P  # 4 columns of [128] query positions per core
NBKT = 1024  # hash buckets for host-side id routing (low 10 id bits)

F32 = mybir.dt.float32
I32 = mybir.dt.int32

DBG_NO_GATHER = bool(int(os.environ.get("BM25_NO_GATHER", "0")))


def _build_program(qcap, pcap):
    W = QC + QC * qcap + QC * pcap + 2  # myqf | q lists | p lists | biases
    nc = bacc.Bacc(
        "TRN2", target_bir_lowering=False, debug=False, num_devices=NCORES
    )
    qidx = nc.dram_tensor("qidx", [P, QC], I32, kind="ExternalInput").ap()
    pack = nc.dram_tensor("pack", [P, W], F32, kind="ExternalInput").ap()
    dfs = nc.dram_tensor("dfs", [VOCAB, 1], F32, kind="ExternalInput").ap()
    out = nc.dram_tensor("out", [1, 1], F32, kind="ExternalOutput").ap()

    with tile.TileContext(nc) as tc, ExitStack() as ctx:
        spool = ctx.enter_context(tc.tile_pool(name="small", bufs=1))
        dpool = ctx.enter_context(tc.tile_pool(name="dummy", bufs=2))

        # gather indices first: they head the longest dependency chain
        t_qidx = spool.tile([P, QC], I32)
        nc.sync.dma_start(out=t_qidx[:], in_=qidx[:], single_packet=True)
        t_pack = spool.tile([P, W], F32)
        nc.sync.dma_start(out=t_pack[:], in_=pack[:])

        # dummy warm gather: absorbs the SWDGE ucode warm-up / library load
        # before the real gathers (its zero offsets need no input data)
        zofs = spool.tile([16, 1], I32)
        nc.gpsimd.memset(zofs[:], 0)
        wdum = spool.tile([16, 1], F32)
        nc.gpsimd.indirect_dma_start(
            out=wdum[:], out_offset=None, in_=dfs[:],
            in_offset=bass.IndirectOffsetOnAxis(ap=zofs[:], axis=0),
        )

        # indirect gather of dfs at my 512 ids.  SWDGE processes one index
        # per partition (channel) per op, so one op per column.
        dfsg = spool.tile([P, QC], F32)
        if DBG_NO_GATHER:
            nc.gpsimd.memset(dfsg[:], 500.0)
        else:
            for k in range(QC):
                nc.gpsimd.indirect_dma_start(
                    out=dfsg[:, k : k + 1],
                    out_offset=None,
                    in_=dfs[:],
                    in_offset=bass.IndirectOffsetOnAxis(
                        ap=t_qidx[:, k : k + 1], axis=0
                    ),
                )

        myqf = t_pack[:, 0:QC]
        qtf = spool.tile([P, QC], F32)
        ptf = spool.tile([P, QC], F32)
        for k in range(QC):
            off = QC + k * qcap
            dq = dpool.tile([P, qcap], F32, tag="dq")
            nc.vector.tensor_scalar(
                out=dq[:],
                in0=t_pack[:, off : off + qcap],
                scalar1=myqf[:, k : k + 1],
                scalar2=None,
                op0=mybir.AluOpType.is_equal,
                op1=mybir.AluOpType.add,
                accum_out=qtf[:, k : k + 1],
            )
        for k in range(QC):
            off = QC + QC * qcap + k * pcap
            dp = dpool.tile([P, pcap], F32, tag="dp")
            nc.vector.tensor_scalar(
                out=dp[:],
                in0=t_pack[:, off : off + pcap],
                scalar1=myqf[:, k : k + 1],
                scalar2=None,
                op0=mybir.AluOpType.is_equal,
                op1=mybir.AluOpType.add,
                accum_out=ptf[:, k : k + 1],
            )

        # term1/qtf = 1/(K3 + qtf)
        ra = spool.tile([P, QC], F32)
        nc.vector.tensor_scalar(
            out=ra[:], in0=qtf[:], scalar1=float(K3), scalar2=None,
            op0=mybir.AluOpType.add,
        )
        nc.vector.reciprocal(ra[:], ra[:])

        # term2/K1 = ptf / (ptf + C2)   (exact 0 when ptf == 0)
        rb = spool.tile([P, QC], F32)
        nc.vector.tensor_scalar(
            out=rb[:], in0=ptf[:], scalar1=float(C2), scalar2=None,
            op0=mybir.AluOpType.add,
        )
        nc.vector.reciprocal(rb[:], rb[:])
        t2 = spool.tile([P, QC], F32)
        nc.vector.tensor_mul(t2[:], ptf[:], rb[:])
        w = spool.tile([P, QC], F32)
        nc.vector.tensor_mul(w[:], ra[:], t2[:])

        # term3*ln2 = ln(N+0.5 - dfs) - ln(dfs + 0.5), pipelined per column
        la = spool.tile([P, QC], F32)
        lb = spool.tile([P, QC], F32)
        t3 = spool.tile([P, QC], F32)
        w2 = spool.tile([P, QC], F32)
        for k in range(QC):
            nc.scalar.activation(
                la[:, k : k + 1], dfsg[:, k : k + 1],
                mybir.ActivationFunctionType.Ln,
                bias=t_pack[:, W - 2 : W - 1], scale=-1.0,
            )
            nc.scalar.activation(
                lb[:, k : k + 1], dfsg[:, k : k + 1],
                mybir.ActivationFunctionType.Ln,
                bias=t_pack[:, W - 1 : W], scale=1.0,
            )
            nc.vector.tensor_sub(
                t3[:, k : k + 1], la[:, k : k + 1], lb[:, k : k + 1]
            )
            nc.vector.tensor_mul(
                w2[:, k : k + 1], w[:, k : k + 1], t3[:, k : k + 1]
            )
        # full reduce on gpsimd -> single-descriptor store
        # (a [128,1] store's completion semaphore costs ~6us more)
        s = spool.tile([1, 1], F32)
        nc.gpsimd.tensor_reduce(
            out=s[:], in_=w2[:],
            axis=mybir.AxisListType.XYZWC, op=mybir.AluOpType.add,
        )
        nc.sync.dma_start(out=out[:], in_=s[:])

    nc.compile()
    return nc


_NC_CACHE = {}


def _get_program(qcap, pcap):
    key = (qcap, pcap)
    if key not in _NC_CACHE:
        _NC_CACHE[key] = _build_program(qcap, pcap)
    return _NC_CACHE[key]


def _roundcap(n):
    return max(8, int(-(-int(n) // 4) * 4))


def _bucket_table(ids, b, cnt, cap, pad):
    order = np.argsort(b, kind="stable")
    ofs = np.arange(ids.size) - np.repeat(np.cumsum(cnt) - cnt, cnt)
    tab = np.full((NBKT, cap), pad, np.float32)
    tab[b[order], ofs] = ids[order].astype(np.float32)
    return tab


def make_in_maps(query_ids, passage_ids, dfs):
    q = np.ascontiguousarray(np.asarray(query_ids).reshape(-1).astype(np.int32))
    p = np.ascontiguousarray(np.asarray(passage_ids).reshape(-1).astype(np.int32))
    d = np.ascontiguousarray(np.asarray(dfs, dtype=np.float32).reshape(VOCAB, 1))

    qb = q & (NBKT - 1)
    pb = p & (NBKT - 1)
    qcnt = np.bincount(qb, minlength=NBKT)
    pcnt = np.bincount(pb, minlength=NBKT)
    qcap = _roundcap(qcnt.max())
    pcap = _roundcap(pcnt.max())
    qtab = _bucket_table(q, qb, qcnt, qcap, -1.0)
    ptab = _bucket_table(p, pb, pcnt, pcap, -2.0)

    in_maps = []
    for c in range(NCORES):
        qc = np.ascontiguousarray(q[c::NCORES].reshape(P, QC))
        bk = qc & (NBKT - 1)
        pack = np.ascontiguousarray(
            np.concatenate(
                [
                    qc.astype(np.float32),
                    qtab[bk].reshape(P, QC * qcap),
                    ptab[bk].reshape(P, QC * pcap),
                    np.full((P, 1), N_DOCS + 0.5, np.float32),
                    np.full((P, 1), 0.5, np.float32),
                ],
                axis=1,
            )
        )
        in_maps.append({"qidx": qc, "pack": pack, "dfs": d})
    return in_maps, qcap, pcap


def kernel(query_ids, passage_ids, dfs, **run_kwargs):
    in_maps, qcap, pcap = make_in_maps(query_ids, passage_ids, dfs)
    nc = _get_program(qcap, pcap)
    # retry on a non-finite total: a degraded launch (observed once, with
    # the whole device ~20% throttled) can garble a gather beat and feed
    # Ln a negative argument; re-running on a recovered device is exact
    for _attempt in range(3):
        res = run_bass_kernel_spmd(
            nc, in_maps, core_ids=list(range(NCORES)), **run_kwargs
        )
        total = sum(float(r["out"][0, 0]) for r in res.results)
        if math.isfinite(total):
            break
    out = np.array([total * K1 * INV_LN2], dtype=np.float32)
    kernel.last_results = res
    return out
